# revision 19
# baseline (speedup 1.0000x reference)
"""HM-GRU v2 Trainium2 Bass kernel.

B=256, T=256, L=2, IN=256, H=256. Data-parallel over batch: 8 cores x 32.

Device layout is feature-major: hidden state h lives as [128 partitions,
(layer, jtile, batch)] where feature f = j*128 + p. Per (t, layer) stage:
  - gate the recurrent inputs with per-(batch,t) 0/1 masks (one tensor_tensor)
  - 36 matmuls accumulate gi_rz+gh_rz into one PSUM bank and gi_n / gh_n
    into a second bank (weights stationary, bf16, N=32)
  - sigmoid/tanh on the scalar engine, the rest of the GRU cell on DVE
  - flush-select via copy_predicated
x projections are folded into the per-stage matmuls; the b0/b1 input gates
and the x transpose to feature-major are precomputed on host. Masks are
DMA-broadcast from compact [T, ...] rows (partition-stride-0 APs).
Output is written feature-major to DRAM and reassembled on host.
"""

import os
import sys
import numpy as np

sys.path.insert(0, "/opt/trn_rl_repo")

B, T, L, IN, H = 256, 256, 2, 256, 256
NCORES = 8
BL = B // NCORES  # 32 batch per core
TBDEF = 32        # time block

_CACHE = {}


# ---------------------------------------------------------------- device build

def _patch_tile_drain():
    """walrus in this env only allows one sync-wait per instruction; split
    multi-wait instructions into single-wait nops + the instruction, and do
    the same for the TileContext exit-drain."""
    import concourse.mybir as mybir
    from concourse.tile import TileContext, ScopedClock

    if getattr(TileContext, "_drain_patched", False):
        return

    def _add_instruction(self, inst):
        si = getattr(inst, "sync_info", None)
        if si is not None and si.on_wait and len(si.on_wait) > 1:
            waits = list(si.on_wait)
            upd = list(si.on_update) if si.on_update else []
            for w in waits[:-1]:
                nop = mybir.InstNoOp(
                    name=self.nc.get_next_instruction_name(),
                    engine=inst.engine,
                    sync_info=mybir.SyncInfo(on_wait=[w], on_update=[]),
                )
                self.nc.register_instruction(nop, overwrite=True)
                self.nc.cur_bb.bb.add_instruction(nop)
            inst.sync_info = mybir.SyncInfo(on_wait=[waits[-1]], on_update=upd)
        self.nc.register_instruction(inst, overwrite=True)
        self.nc.cur_bb.bb.add_instruction(inst)

    TileContext._add_instruction = _add_instruction

    def _drain_and_barrier(self, tick_clock, wait_clock):
        drain_inst = self.nc.sync.drain()
        wait_clock.add_sem_waits(
            drain_inst.ins, ScopedClock({None: tick_clock.global_clock})
        )
        si = drain_inst.ins.sync_info
        if si is not None and si.on_wait and len(si.on_wait) > 1:
            waits = list(si.on_wait)
            upd = list(si.on_update) if si.on_update else []
            drain_inst.ins.sync_info = mybir.SyncInfo(
                on_wait=[waits[0]], on_update=upd
            )
            for w in waits[1:]:
                nop = self.nc.sync.nop(nofuse=True)
                nop.ins.sync_info = mybir.SyncInfo(on_wait=[w], on_update=[])
        self.nc.all_engine_barrier()
        popped = self.nc._tile_sem_poison_stack.pop()
        assert popped is self._sem_poison
        self.nc.clear_and_free_semaphores(list(self.sems.allocated().values()))
        self.nc.all_engine_barrier()

    TileContext._drain_and_barrier = _drain_and_barrier
    TileContext._drain_patched = True


def build_bass(t_run=T, tb=TBDEF):
    import concourse.bass as bass
    import concourse.mybir as mybir
    from concourse.tile import TileContext
    from concourse.tile import add_dep_helper

    _patch_tile_drain()

    nb = t_run // tb
    bf16 = mybir.dt.bfloat16
    f32 = mybir.dt.float32
    Alu = mybir.AluOpType
    Act = mybir.ActivationFunctionType

    nc = bass.Bass()
    xt_d = nc.declare_dram_parameter("xt", [128, 4, t_run, BL], bf16, isOutput=False)
    gm_d = nc.declare_dram_parameter("gm", [t_run, 2, 128], bf16, isOutput=False)
    u8 = mybir.dt.uint8
    sm_d = nc.declare_dram_parameter("sm", [t_run, 2, 64], u8, isOutput=False)
    wts_d = nc.declare_dram_parameter("wts", [6, 128, 2, 768], bf16, isOutput=False)
    bmm_d = nc.declare_dram_parameter("bmm", [2, 2, 4, 128], bf16, isOutput=False)
    ind_d = nc.declare_dram_parameter("ind", [4, 128], bf16, isOutput=False)
    hinit_d = nc.declare_dram_parameter("hinit", [128, 128], bf16, isOutput=False)
    out_d = nc.declare_dram_parameter(
        "out", [2, 2, 128, t_run, BL], bf16, isOutput=True
    )

    def bcast(d_ap, t0):
        src = d_ap[t0 : t0 + tb]
        return bass.AP(
            tensor=src.tensor, offset=src.offset, ap=[[0, 128]] + list(src.ap)
        )

    with TileContext(nc) as tc:
        with (
            tc.tile_pool(name="wpool", bufs=1) as wpool,
            tc.tile_pool(name="xpool", bufs=2) as xpool,
            tc.tile_pool(name="mpool", bufs=2) as mpool,
            tc.tile_pool(name="spool", bufs=2) as spool,
            tc.tile_pool(name="rpool", bufs=4) as rpool,
            tc.tile_pool(name="tpool", bufs=4) as tpool,
            tc.tile_pool(name="psum", bufs=3, space="PSUM") as pspool,
            tc.tile_pool(name="psum2", bufs=2, space="PSUM") as pspool2,
        ):
            # persistent weights / bias-matmul constants
            w_sb = []
            for i in range(6):
                w = wpool.tile([128, 2, 768], bf16, tag=f"w{i}")
                nc.sync.dma_start(out=w, in_=wts_d[i])
                w_sb.append(w)
            bmm_sb = wpool.tile([4, 2, 2, 128], bf16, tag="bmm")
            nc.sync.dma_start(
                out=bmm_sb,
                in_=bmm_d[:, :, :, :].rearrange("l c k p -> k l c p"),
            )
            ind_sb = wpool.tile([4, 128], bf16, tag="ind")
            nc.sync.dma_start(out=ind_sb, in_=ind_d[:, :])

            def lhs(w, k, m):
                return w[:, k, m * 128 : (m + 1) * 128]

            class Stage:
                pass

            prev_stg = None
            for ib in range(nb):
                t0 = ib * tb
                xb = xpool.tile([128, 4, tb, BL], bf16, tag="xb")
                nc.sync.dma_start(out=xb, in_=xt_d[:, :, t0 : t0 + tb, :])
                gm_sb = mpool.tile([128, tb, 2, 128], bf16, tag="gm")
                nc.sync.dma_start(out=gm_sb, in_=bcast(gm_d, t0))
                sm_sb = mpool.tile([128, tb, 2, 64], u8, tag="sm")
                nc.sync.dma_start(out=sm_sb, in_=bcast(sm_d, t0))

                stg = spool.tile([128, (tb + 1) * 128], bf16, tag="stg")
                if ib == 0:
                    nc.sync.dma_start(out=stg[:, 0:128], in_=hinit_d[:, :])
                else:
                    nc.vector.tensor_copy(
                        stg[:, 0:128], prev_stg[:, tb * 128 : (tb + 1) * 128]
                    )

                stages = [(ti, lx) for ti in range(tb) for lx in (0, 1)]

                def prefetch(si):
                    """bias/x/self matmuls + self-gate for stage si.
                    Depends on nothing later than chain(si-2)."""
                    ti, lx = stages[si]
                    st = Stage()
                    st.rgtp = None
                    st.bank_rz = pspool.tile([128, 128], f32, tag="bank_rz")
                    st.bank_n = pspool.tile([128, 128], f32, tag="bank_n")
                    w_x, w_self = w_sb[lx * 3], w_sb[lx * 3 + 2]
                    # bias injection (rank-4): bank = bmm.T @ ind
                    nc.tensor.matmul(st.bank_rz, bmm_sb[:, lx, 0, :], ind_sb,
                                     start=True, stop=False,
                                     skip_group_check=True)
                    nc.tensor.matmul(st.bank_n, bmm_sb[:, lx, 1, :], ind_sb,
                                     start=True, stop=False,
                                     skip_group_check=True)
                    # self-gate (h_self(t-1) * omdd) on DVE, early
                    st.rg_self = rpool.tile([128, 64], bf16, tag="rg_self")
                    nc.vector.tensor_tensor(
                        st.rg_self, stg[:, ti * 128 + lx * 64 :
                                        ti * 128 + lx * 64 + 64],
                        gm_sb[:, ti, lx, 0:64], op=Alu.mult
                    )
                    for m in range(4):
                        o = st.bank_rz[:, m * 32 : (m + 1) * 32]
                        nc.tensor.matmul(o, lhs(w_x, 0, m), xb[:, lx * 2, ti, :],
                                         start=False, stop=False,
                                         skip_group_check=True)
                        nc.tensor.matmul(o, lhs(w_x, 1, m), xb[:, lx * 2 + 1, ti, :],
                                         start=False, stop=False,
                                         skip_group_check=True)
                        for k in (0, 1):
                            nc.tensor.matmul(
                                o, lhs(w_self, k, m),
                                st.rg_self[:, k * 32 : (k + 1) * 32],
                                start=False, stop=False, skip_group_check=True)
                    for j in (0, 1):
                        m = 4 + j
                        o = st.bank_n[:, j * 32 : (j + 1) * 32]
                        nc.tensor.matmul(o, lhs(w_x, 0, m), xb[:, lx * 2, ti, :],
                                         start=False, stop=False,
                                         skip_group_check=True)
                        nc.tensor.matmul(o, lhs(w_x, 1, m), xb[:, lx * 2 + 1, ti, :],
                                         start=False, stop=False,
                                         skip_group_check=True)
                        og = st.bank_n[:, 64 + j * 32 : 64 + (j + 1) * 32]
                        for k in (0, 1):
                            nc.tensor.matmul(
                                og, lhs(w_self, k, m),
                                st.rg_self[:, k * 32 : (k + 1) * 32],
                                start=False, stop=False, skip_group_check=True)
                    return st

                def emit_top(si, st):
                    """top matmuls; for a block-first stage also the
                    top-gate (otherwise rgtp came from the prior chain)."""
                    ti, lx = stages[si]
                    w_top = w_sb[lx * 3 + 1]
                    if st.rgtp is None:
                        # top operand: L0 <- h1(t-1); L1 <- h0(t)
                        toff = ti * 128 + 64 if lx == 0 else (ti + 1) * 128
                        st.rgtp = rpool.tile([128, 64], bf16, tag="rg_top")
                        nc.vector.tensor_tensor(
                            st.rgtp, stg[:, toff : toff + 64],
                            gm_sb[:, ti, lx, 64:128], op=Alu.mult
                        )
                    for m in range(4):
                        o = st.bank_rz[:, m * 32 : (m + 1) * 32]
                        for k in (0, 1):
                            nc.tensor.matmul(
                                o, lhs(w_top, k, m),
                                st.rgtp[:, k * 32 : (k + 1) * 32],
                                start=False, stop=(m == 3 and k == 1),
                                skip_group_check=True)
                    for j in (0, 1):
                        m = 4 + j
                        o = st.bank_n[:, j * 32 : (j + 1) * 32]
                        for k in (0, 1):
                            nc.tensor.matmul(
                                o, lhs(w_top, k, m),
                                st.rgtp[:, k * 32 : (k + 1) * 32],
                                start=False, stop=(j == 1 and k == 1),
                                skip_group_check=True)

                def emit_chain(si, st, nxt):
                    ti, lx = stages[si]
                    hg = st.rg_self
                    smask = sm_sb[:, ti, lx, :]
                    if nxt is not None:
                        nti, nlx = stages[si + 1]
                        ddm = gm_sb[:, nti, nlx, 64:128]
                        amat = rpool.tile([128, 64], bf16, tag="amat")
                        nc.vector.tensor_tensor(amat, hg, ddm, op=Alu.mult)
                    rz = tpool.tile([128, 128], bf16, tag="rz")
                    nc.scalar.activation(rz, st.bank_rz, Act.Sigmoid)
                    z = rz[:, 64:128]
                    rt = tpool.tile([128, 64], bf16, tag="rt")
                    nc.vector.tensor_tensor(
                        rt, st.bank_n[:, 64:128], rz[:, 0:64], op=Alu.mult
                    )
                    narg = pspool2.tile([128, 64], f32, tag="narg")
                    narg_inst = nc.vector.tensor_tensor(
                        narg, st.bank_n[:, 0:64], rt, op=Alu.add
                    )
                    omz = tpool.tile([128, 64], bf16, tag="omz")
                    nc.scalar.activation(omz, z, Act.Identity,
                                         bias=1.0, scale=-1.0)
                    nt = tpool.tile([128, 64], bf16, tag="nt")
                    nc.scalar.activation(nt, narg, Act.Tanh)
                    pt_inst = None
                    if nxt is not None:
                        # chain tail producing next stage's gated top operand:
                        # rgtp = select(c, A, nt*(dd*(1-z)) + z*A)
                        omzdd = tpool.tile([128, 64], bf16, tag="omzdd")
                        oz_inst = nc.vector.tensor_tensor(
                            omzdd, omz, ddm, op=Alu.mult)
                        add_dep_helper(oz_inst.ins, narg_inst.ins, sync=False,
                                       reason="keep chain rt->narg first")
                        w22 = tpool.tile([128, 64], bf16, tag="w22")
                        w22_inst = nc.vector.tensor_tensor(
                            w22, z, amat, op=Alu.mult)
                        add_dep_helper(w22_inst.ins, narg_inst.ins, sync=False,
                                       reason="keep chain rt->narg first")
                        ha2 = tpool.tile([128, 64], bf16, tag="ha2")
                        nc.vector.tensor_tensor(ha2, nt, omzdd, op=Alu.mult)
                        pre = rpool.tile([128, 64], bf16, tag="pre")
                        nc.vector.tensor_tensor(pre, ha2, w22, op=Alu.add)
                        pt_inst = nc.vector.copy_predicated(pre, smask, amat)
                        nxt.rgtp = pre
                    # h'' materialization for output/self-gate (off-chain)
                    w2 = tpool.tile([128, 64], bf16, tag="w2")
                    nc.gpsimd.tensor_tensor(w2, z, hg, op=Alu.mult)
                    ha = tpool.tile([128, 64], bf16, tag="ha")
                    nc.gpsimd.tensor_tensor(ha, nt, omz, op=Alu.mult)
                    hoff = (ti + 1) * 128 + lx * 64
                    hslot = stg[:, hoff : hoff + 64]
                    nc.gpsimd.tensor_tensor(hslot, ha, w2, op=Alu.add)
                    p2_inst = nc.vector.copy_predicated(hslot, smask, hg)
                    if pt_inst is not None:
                        add_dep_helper(
                            p2_inst.ins, pt_inst.ins, sync=False,
                            reason="stg patch after chain-critical predTop",
                        )

                st = prefetch(0)
                for si in range(len(stages)):
                    emit_top(si, st)
                    nxt = prefetch(si + 1) if si + 1 < len(stages) else None
                    emit_chain(si, st, nxt)
                    st = nxt

                stg_view = stg[:, 128:].rearrange(
                    "p (t c b) -> p t c b", t=tb, c=4
                )
                for l in (0, 1):
                    for j in (0, 1):
                        nc.sync.dma_start(
                            out=out_d[l, j, :, t0 : t0 + tb, :],
                            in_=stg_view[:, :, l * 2 + j, :],
                        )
                prev_stg = stg
    return nc


# ------------------------------------------------------------------ host side

def _bcast_rows(sig, bs, t_run):
    """sig [B, T] -> [t_run, 64]: value per (t, j, b) replicated over j."""
    s = sig[bs, :t_run].T                      # [T, BL]
    return np.repeat(s[:, None, :], 2, axis=1).reshape(t_run, 2 * BL)


def prep_host(x0, x1, hx0, hx1, W_ih0, W_hh0, b_ih0, b_hh0,
              W_ih1, W_hh1, b_ih1, b_hh1, dx, dx_layer_zero, t_run=T):
    import ml_dtypes

    f = np.float32
    bf = ml_dtypes.bfloat16
    dxf = np.asarray(dx).astype(f)
    db0 = np.asarray(dx_layer_zero).astype(f)          # [B, T]
    db1 = dxf[:, 0, :]
    z1 = np.zeros((B, 1), f)
    d0 = np.concatenate([z1, dxf[:, 0, :-1]], axis=1)
    d1 = np.concatenate([z1, dxf[:, 1, :-1]], axis=1)
    omdd0, omdd1 = 1.0 - d0, 1.0 - d1
    c0 = ((db0 + d0) == 0.0).astype(f)
    c1 = ((db1 + d1) == 0.0).astype(f)

    x0g = np.asarray(x0, f) * db0[:, :, None]          # [B, T, IN]
    x1g = np.asarray(x1, f) * db1[:, :, None]

    # weights (shared): lhsT tiles [6, 128, 2, 768]; value = W[M, k*128+p]
    wts = np.empty((6, 128, 2, 768), f)
    for lx, (Wih, Whh) in enumerate(((W_ih0, W_hh0), (W_ih1, W_hh1))):
        Wih = np.asarray(Wih, f)
        Whh = np.asarray(Whh, f)
        for pi, Wm in enumerate((Wih[:, :IN], Wih[:, IN:], Whh)):
            # Wm [768, 256]; lhsT[k*128+p, M] = Wm[M, k*128+p]
            wts[lx * 3 + pi] = Wm.T.reshape(2, 128, 768).transpose(1, 0, 2)
    wts = wts.astype(bf)

    # bias-matmul constants: bmm [2(layer), 2(bank), 4, 128], ind [4, 128]
    b_ih = [np.asarray(b_ih0, f), np.asarray(b_ih1, f)]
    b_hh = [np.asarray(b_hh0, f), np.asarray(b_hh1, f)]
    bmm = np.zeros((2, 2, 4, 128), f)
    for lx in 0, 1:
        bmm[lx, 0] = (b_ih[lx] + b_hh[lx])[:512].reshape(4, 128)
        bmm[lx, 1, 0:2] = b_ih[lx][512:].reshape(2, 128)
        bmm[lx, 1, 2:4] = b_hh[lx][512:].reshape(2, 128)
    ind = np.zeros((4, 128), f)
    for k in range(4):
        ind[k, k * 32 : (k + 1) * 32] = 1.0

    hx = [np.asarray(hx0, f), np.asarray(hx1, f)]

    per_core = []
    for c in range(NCORES):
        bs = slice(c * BL, (c + 1) * BL)
        xt = np.empty((128, 4, t_run, BL), f)
        for lx, xg in enumerate((x0g, x1g)):
            xcT = xg[bs, :t_run].transpose(2, 1, 0)    # [IN, T, BL]
            xt[:, lx * 2 + 0] = xcT[:128]
            xt[:, lx * 2 + 1] = xcT[128:]
        gm = np.empty((t_run, 2, 128), f)
        gm[:, 0, 0:64] = _bcast_rows(omdd0, bs, t_run)
        gm[:, 0, 64:128] = _bcast_rows(d0, bs, t_run)
        gm[:, 1, 0:64] = _bcast_rows(omdd1, bs, t_run)
        gm[:, 1, 64:128] = _bcast_rows(db1, bs, t_run)
        sm = np.empty((t_run, 2, 64), f)
        sm[:, 0] = _bcast_rows(c0, bs, t_run)
        sm[:, 1] = _bcast_rows(c1, bs, t_run)
        hinit = np.empty((128, 128), f)
        for lx in 0, 1:
            hcT = hx[lx][bs].T.reshape(2, 128, BL)     # [j, p, b]
            hinit[:, lx * 64 : lx * 64 + 32] = hcT[0]
            hinit[:, lx * 64 + 32 : (lx + 1) * 64] = hcT[1]
        per_core.append(
            dict(
                xt=xt.astype(bf),
                gm=gm.astype(bf),
                sm=sm.astype(np.uint8),
                wts=wts,
                bmm=bmm.astype(bf),
                ind=ind.astype(bf),
                hinit=hinit.astype(bf),
            )
        )
    return per_core


def unshard(results, t_run=T):
    """results: list of per-core dicts with 'out' [2, 2, 128, T, BL] bf16."""
    out = np.empty((L, B, t_run, H), np.float32)
    for c, res in enumerate(results):
        o = np.asarray(res["out"], np.float32)          # [2, 2, 128, T, BL]
        # out[l, c*BL+b, t, j*128+p] = o[l, j, p, t, b]
        oc = o.transpose(0, 4, 3, 1, 2).reshape(L, BL, t_run, H)
        out[:, c * BL : (c + 1) * BL] = oc
    return out


def _install_profile_shim():
    """Register the NTFF profile hook (missing antenv.axon_hooks module in
    this image) so run_bass_kernel_spmd(trace=True) yields exec_time_ns."""
    import types

    if "antenv.axon_hooks" in sys.modules:
        return
    try:
        mod = types.ModuleType("antenv.axon_hooks")
        mod._hook = None
        mod.set_axon_ntff_profile_hook = lambda h: setattr(mod, "_hook", h)
        mod.get_axon_ntff_profile_hook = lambda: mod._hook
        sys.modules["antenv.axon_hooks"] = mod
        if "/root/.axon_site" not in sys.path:
            sys.path.insert(0, "/root/.axon_site")
        from trn_agent_boot.trn_boot import _ntff_profile_via_ctypes

        mod._hook = _ntff_profile_via_ctypes("/opt/axon/libaxon_pjrt.so")
        import concourse.bass_utils as bu

        bu.upload_artifacts = lambda tmpdir: f"local:{tmpdir}"
    except Exception:
        pass


def run_device(per_core, t_run=T, tb=TBDEF, trace=False, **kw):
    if trace:
        _install_profile_shim()
    from concourse.bass_utils import run_bass_kernel_spmd

    key = (t_run, tb)
    if key not in _CACHE:
        _CACHE[key] = build_bass(t_run, tb)
    nc = _CACHE[key]
    return run_bass_kernel_spmd(
        nc, per_core, list(range(NCORES)), trace=trace, **kw
    )


def kernel(x0, x1, hx0, hx1, W_ih0, W_hh0, b_ih0, b_hh0,
           W_ih1, W_hh1, b_ih1, b_hh1, dx, dx_layer_zero):
    per_core = prep_host(
        x0, x1, hx0, hx1, W_ih0, W_hh0, b_ih0, b_hh0,
        W_ih1, W_hh1, b_ih1, b_hh1, dx, dx_layer_zero
    )
    res = run_device(per_core)
    return unshard(res.results)
